# revision 50
# baseline (speedup 1.0000x reference)
"""Bass/Tile kernel for causal multi-head attention block (nn_BlankAttention).

Sharding: 8 cores = 2 batches x 4 head-groups (4 heads each).
Each core computes, for its batch b and heads hg*4..hg*4+3:
  qkv projection, causal attention, partial output projection
  y_part = attn_out @ w_out_slice.  Host sums the 4 partials per batch.

Design notes (measured on HW: fp32r matmul ~216-227ns per 512 free rows
at full clock; fp32r below 256 free rows pays 4x, bf16 is rate-flat; the
part shows +-10% run-to-run DVFS variance):
  * everything matmul-heavy runs bf16 (same matmul rate as fp32r at full
    clock on this part, half the DMA bytes and SBUF); PSUM accumulates in
    f32.  DVE tensor ops accept mixed bf16/f32 inputs.
  * projection is a single full-K pass: 16-matmul PSUM chains per output
    tile, one evacuation, x streamed once (13.5MB total input DMA).  The
    opening chunk is DMA-bandwidth-bound, so chunk 0 opens with the
    V-groups while the wqk weights stream behind.
  * attention for l-tile i is interleaved into projection chunk i+1's
    matmul stream: exp latency hides under projection matmuls, the PE
    never idles, and its p-state stays at max clock.
  * score tiles for key-tile pairs share one 2-bank PSUM tile [128,1024]:
    one exp per pair (split only for diagonal pairs so the lo-skipped
    PSUM region is never read); denominator matmuls run once per FOUR
    pairs on a running DVE-accumulated sum of pair-sums.
  * denominator matmul uses an all-ones [128,128] stationary, so the sum
    lands pre-broadcast across partitions: no [1,512]-out matmul (slow on
    this part) and no separate K=1 broadcast matmul.
  * softmax: reciprocal_approx_fast (~0.7us, 18 bits) then one DVE
    multiply straight out of the oacc PSUM bank into OT.
  * output projection is deferred to the post-projection phase (PSUM
    banks: overlap = ps2+sc4+oacc1+ssum1, tail = sc4+oacc1+ssum1+yp2) and
    woven one row-block per ~6 attention steps between l-tile 3's
    attention; y leaves bf16 as 16 fat [128,2048] DMAs (host upcasts).

Per-core DRAM tensors (all bf16; host casts in, upcasts/sums y out):
  xt    [2048, 2048] x[b].T                    (dmodel, tok)
  wqk   [2048, 1024] w_in q/k cols             ([q_h0|k_h0|q_h1|k_h1|...])
  wv    [2048,  512] w_in v cols               ([v_h0|v_h1|v_h2|v_h3])
  wout  [ 512, 2048] w_out rows for the 4 heads (head-major)
  maskt [n_u,  128, 512] mask tiles, 1.0 = allowed, 0.0 = masked
  ones  [ 128,  128]
  y     [2048, 2048] output partial (tok, dmodel)

schedule: list over l-tile i (4 tiles of 512 queries) of list of
  (j, mask_idx, lo) -- key tiles (128 keys); mask_idx -1 = no mask; lo =
  leading fully-masked query columns (only the AV matmul skips them;
  scores/exp/sums run full width so no uninitialized PSUM is ever read).
"""

import numpy as np
import concourse.bass as bass
import concourse.tile as tile
from concourse import bacc, mybir

S = 2048
DM = 2048
NHL = 4          # heads per core
DH = 128
SCALE = 1.0 / (DH ** 0.5)

F32 = mybir.dt.float32
F32R = mybir.dt.float32r
BF16 = mybir.dt.bfloat16
EXP = mybir.ActivationFunctionType.Exp


def build_nc(schedule, n_masks):
    nc = bacc.Bacc("TRN2", target_bir_lowering=False, debug=False, num_devices=8)
    xt_d = nc.dram_tensor("xt", [DM, S], BF16, kind="ExternalInput").ap()
    wqk_d = nc.dram_tensor("wqk", [DM, 2 * NHL * DH], BF16, kind="ExternalInput").ap()
    wv_d = nc.dram_tensor("wv", [DM, NHL * DH], BF16, kind="ExternalInput").ap()
    wout_d = nc.dram_tensor("wout", [NHL * DH, DM], BF16, kind="ExternalInput").ap()
    maskt_d = nc.dram_tensor("maskt", [n_masks, 128, 512], BF16, kind="ExternalInput").ap()
    ones_d = nc.dram_tensor("ones", [128, 128], BF16, kind="ExternalInput").ap()
    y_d = nc.dram_tensor("y", [S, DM], BF16, kind="ExternalOutput").ap()

    from collections import deque

    with tile.TileContext(nc) as tc:
        with tc.tile_pool(name="persist", bufs=1) as pp, \
             tc.tile_pool(name="attps", bufs=1, space="PSUM") as aps:
            qkT = pp.tile([128, 8, S], BF16)       # [dh, (2h+isK), tok]
            V = pp.tile([128, 16, NHL * DH], BF16)  # [tok%128, tok//128, vfeat]
            masks = pp.tile([128, n_masks, 512], BF16)
            ones_t = pp.tile([128, 128], BF16)
            OT = pp.tile([128, 4, S], BF16)        # [dh, h, tok], normalized
            wt = pp.tile([128, 4, S], BF16)        # w_out^T [dh, h, od]
            pre = pp.tile([1, 8], F32)

            # pre-zero the ex rotation slots: the first diagonal pairs leave
            # their [:lo] region unwritten and the mask-multiply reads it
            for z in range(2):
                ex0 = pp.tile([128, 1024], BF16, tag="ex", bufs=2, name=f"exz{z}")
                nc.gpsimd.memset(ex0[:], 0.0)

            att_q = deque()
            aux = [None]   # phase-2 PSUM pool (yp), assigned when it opens
            p2 = [None]    # phase-2 SBUF pool (ys, woutT)

            def pop_att(k):
                for _ in range(k):
                    if att_q:
                        att_q.popleft()()

            # ---------- attention machinery ----------
            def att_steps(i, fillers=None):
                """Closures for l-tile i's attention, software-pipelined per
                head: A(p)=scores+exp+mask for pair p; F(p)=pair-add+sums+AV;
                E=evacuate + inline softmax-normalization."""
                js = schedule[i]
                pairs = [tuple(js[k:k + 2]) for k in range(0, len(js), 2)]
                steps = []
                for h in range(4):
                    state = {}

                    def mk_A(h, i, p, pr, state):
                        def A():
                            sc = aps.tile([128, 1024], F32, tag="sc", bufs=2,
                                          name=f"sc{i}_{h}_{p}")
                            ex = pp.tile([128, 1024], BF16, tag="ex", bufs=2,
                                         name=f"ex{i}_{h}_{p}")
                            for half, (j, mi, lo) in enumerate(pr):
                                nc.tensor.matmul(
                                    sc[:, 512 * half + lo:512 * (half + 1)],
                                    qkT[:, 2 * h + 1, 128 * j:128 * (j + 1)],
                                    qkT[:, 2 * h, 512 * i + lo:512 * (i + 1)],
                                    start=True, stop=True)
                            if any(lo for (j, mi, lo) in pr):
                                # split exp so it never reads the PSUM region
                                # the scores matmul skipped; ex[:lo] holds
                                # zeros/old exps (slots are pre-zeroed), and
                                # the full-width mask-mul zeroes it anyway
                                for half, (j, mi, lo) in enumerate(pr):
                                    nc.scalar.activation(
                                        ex[:, 512 * half + lo:512 * (half + 1)],
                                        sc[:, 512 * half + lo:512 * (half + 1)],
                                        EXP, scale=SCALE)
                            else:
                                nc.scalar.activation(ex[:], sc[:], EXP, scale=SCALE)
                            for half, (j, mi, lo) in enumerate(pr):
                                exh = ex[:, 512 * half:512 * (half + 1)]
                                if mi >= 0:
                                    nc.vector.tensor_mul(exh, exh, masks[:, mi, :])
                            state[p] = ex
                        return A

                    def mk_F(h, i, p, pr, state, first, last):
                        def F():
                            ex = state.pop(p)
                            if first:
                                state['oacc'] = aps.tile(
                                    [128, 512], F32, tag="oacc", bufs=1,
                                    name=f"oacc{i}_{h}")
                                state['ssum'] = aps.tile(
                                    [128, 512], F32, tag="ssum", bufs=1,
                                    name=f"ssum{i}_{h}")
                            oacc, ssum = state['oacc'], state['ssum']
                            exs = pp.tile([128, 512], BF16, tag="exs", bufs=3,
                                          name=f"exs{i}_{h}_{p}")
                            nc.vector.tensor_add(exs[:], ex[:, 0:512], ex[:, 512:1024])
                            acc = state.get('exs')
                            if acc is not None:
                                nc.vector.tensor_add(exs[:], exs[:], acc[:])
                            if p % 4 == 3 or last:
                                # all-ones stationary: denominator lands
                                # broadcast across all 128 partitions
                                nc.tensor.matmul(ssum[:], ones_t[:], exs[:],
                                                 start=(p < 4), stop=last)
                                state['exs'] = None
                            else:
                                state['exs'] = exs
                            for half, (j, mi, lo) in enumerate(pr):
                                nc.tensor.matmul(
                                    oacc[:, lo:], V[:, j, 128 * h:128 * (h + 1)],
                                    ex[:, 512 * half + lo:512 * (half + 1)],
                                    start=(first and half == 0),
                                    stop=(last and half == 1))
                        return F

                    def mk_E(h, i, state):
                        def E():
                            oacc, ssum = state['oacc'], state['ssum']
                            rec = pp.tile([128, 512], F32, tag="rec", bufs=2,
                                          name=f"rec{i}_{h}")
                            nc.vector.reciprocal_approx_fast(out=rec[:], in_=ssum[:])
                            # normalize straight out of PSUM; oacc frees after
                            nc.vector.tensor_mul(OT[:, h, 512 * i:512 * (i + 1)],
                                                 oacc[:], rec[:])
                            if fillers is not None and h == 3:
                                for tt in range(4 * i, 4 * i + 4):
                                    fillers.append(mk_oproj(tt))
                        return E

                    n = len(pairs)
                    A = [mk_A(h, i, p, pr, state) for p, pr in enumerate(pairs)]
                    F = [mk_F(h, i, p, pr, state, p == 0, p == n - 1)
                         for p, pr in enumerate(pairs)]
                    # software pipeline: A0 A1 F0 A2 F1 ... A_{n-1} F_{n-2} F_{n-1}
                    for p in range(n):
                        steps.append(A[p])
                        if p >= 1:
                            steps.append(F[p - 1])
                    steps.append(F[n - 1])
                    steps.append(mk_E(h, i, state))
                return steps

            def mk_oproj(tt):
                """One token row-block: 4 o-groups of 4 PSUM-chained matmuls,
                gathered into [128,2048], one fat DMA out."""
                def G():
                    ys = p2[0].tile([128, S], BF16, tag="ys", bufs=2, name=f"ys{tt}")
                    for o in range(4):
                        yp = aux[0].tile([128, 512], F32, tag="yp", bufs=2,
                                         name=f"yp{tt}_{o}")
                        for h in range(4):
                            nc.tensor.matmul(
                                yp[:], OT[:, h, 128 * tt:128 * (tt + 1)],
                                wt[:, h, 512 * o:512 * (o + 1)],
                                start=(h == 0), stop=(h == 3))
                        if o % 2 == 0:
                            nc.vector.tensor_copy(ys[:, 512 * o:512 * (o + 1)], yp[:])
                        else:
                            nc.scalar.copy(ys[:, 512 * o:512 * (o + 1)], yp[:])
                        nc.sync.dma_start(
                            y_d[128 * tt:128 * (tt + 1), 512 * o:512 * (o + 1)],
                            ys[:, 512 * o:512 * (o + 1)])
                return G

            # ---------- phase 1: projection (single full-K pass, bf16) ----------
            # bf16 operands run at the same matmul rate as fp32r on this part
            # at full clock, so the whole projection goes bf16: half the DMA
            # bytes (the opening chunk is DMA-bandwidth-bound), x streamed
            # once, and the full K=2048 contraction chains in PSUM (no
            # cross-pass DVE adds at all).
            fillers = deque()
            with tc.tile_pool(name="proj", bufs=1) as projp, \
                 tc.tile_pool(name="pps", bufs=1, space="PSUM") as pps:
                wqk_s = projp.tile([128, 16, 2 * NHL * DH], BF16, name="wqk_s")
                wv_s = projp.tile([128, 16, NHL * DH], BF16, name="wv_s")
                for c in range(4):
                    xt_c = projp.tile([128, 16, 512], BF16, tag="xt", bufs=2,
                                      name=f"xt_{c}")
                    if c == 0:
                        # V-group operands (wv+xt, 2MB) stream first so
                        # compute starts before the 2MB wqk lands
                        for a in range(16):
                            nc.sync.dma_start(
                                wv_s[:, a:a + 1, :],
                                wv_d[128 * a:128 * (a + 1), :]
                                .rearrange("(q p) f -> p q f", p=128))
                            nc.sync.dma_start(
                                xt_c[:, a:a + 1, :],
                                xt_d[128 * a:128 * (a + 1), 0:512]
                                .rearrange("(q p) t -> p q t", p=128))
                            if a == 0:
                                nc.sync.dma_start(ones_t[:], ones_d[:])
                                nc.scalar.activation(pre[:], ones_t[0:1, 0:8],
                                                     EXP, scale=1.0)
                        for a in range(8):
                            nc.sync.dma_start(
                                wqk_s[:, 2 * a:2 * a + 2, :],
                                wqk_d[256 * a:256 * (a + 1), :]
                                .rearrange("(q p) f -> p q f", p=128))
                    else:
                        for (a, b) in ((0, 8), (8, 16)):
                            nc.sync.dma_start(
                                xt_c[:, a:b, :],
                                xt_d[128 * a:128 * b, 512 * c:512 * (c + 1)]
                                .rearrange("(q p) t -> p q t", p=128))
                        if c == 1:
                            # masks aren't read until the first attention
                            # steps; keep them out of the DMA-saturated
                            # opening window
                            nc.sync.dma_start(
                                masks[:], maskt_d.rearrange("u p c -> p u c"))
                    n_steps = len(att_q)
                    order = [8, 9, 10, 11, 0, 1, 2, 3, 4, 5, 6, 7] \
                        if c == 0 else list(range(12))
                    for g in order:
                        if g < 8:
                            ps = pps.tile([128, 512], F32, tag="ps", bufs=2,
                                          name=f"ps{c}_{g}")
                            for dq in range(16):
                                nc.tensor.matmul(
                                    ps[:], wqk_s[:, dq, 128 * g:128 * (g + 1)],
                                    xt_c[:, dq, :],
                                    start=(dq == 0), stop=(dq == 15))
                            dst = qkT[:, g, 512 * c:512 * (c + 1)]
                        else:
                            tt = g - 8
                            ps = pps.tile([128, 512], F32, tag="ps", bufs=2,
                                          name=f"psv{c}_{g}")
                            for dq in range(16):
                                nc.tensor.matmul(
                                    ps[:], xt_c[:, dq, 128 * tt:128 * (tt + 1)],
                                    wv_s[:, dq, :],
                                    start=(dq == 0), stop=(dq == 15))
                            dst = V[:, 4 * c + tt, :]
                        if g % 2 == 0:
                            nc.vector.tensor_copy(dst, ps[:])
                        else:
                            nc.scalar.copy(dst, ps[:])
                        pop_att(-(-n_steps // 12) if n_steps else 0)
                    if c == 1:
                        # stream w_out^T mid-projection so the output
                        # projection can start the moment phase 2 opens
                        for hh in range(4):
                            nc.sync.dma_start(
                                wt[:, hh:hh + 1, :],
                                wout_d[128 * hh:128 * (hh + 1), :]
                                .rearrange("(h p) o -> p h o", p=128))
                    if c < 3:
                        att_q.extend(att_steps(c, fillers=fillers))

            # ---------- phase 2: l-tile 3 + output projection ----------
            # att_q still holds the tail of l-tile 2's steps; fillers holds
            # oproj units appended by completed E steps (OT rows guaranteed
            # issued before their consumers).
            with tc.tile_pool(name="auxps", bufs=1, space="PSUM") as aux_pool, \
                 tc.tile_pool(name="p2sb", bufs=1) as p2sb:
                aux[0] = aux_pool
                p2[0] = p2sb
                att_q.extend(att_steps(3, fillers=fillers))
                # weave: ~6 attention steps per oproj filler so the filler
                # supply lasts through the whole attention stream (fillers
                # arrive progressively as E steps complete)
                while att_q or fillers:
                    pop_att(6)
                    if fillers:
                        fillers.popleft()()
    nc.compile()
    return nc


def derive_schedule(mask):
    """mask: [S, S] bool, mask[l, L] True = masked (key L not visible to query l).

    Returns (schedule, mask_tiles):
      schedule[i] = list of (j, mask_idx, lo) for l-tile i; mask_idx -1 = all
      allowed; lo = leading fully-masked query columns (only the AV matmul
      skips them).
      mask_tiles: [n_u, 128, 512] float32, allowed=1.0
    """
    schedule = []
    uniq = {}
    tiles = []
    for i in range(4):
        row = []
        for j in range(16):
            blk = mask[512 * i:512 * (i + 1), 128 * j:128 * (j + 1)]  # [l 512, L 128]
            if blk.all():
                continue  # fully masked -> skip tile
            if not blk.any():
                row.append((j, -1, 0))
                continue
            t = (~blk.T).astype(np.float32)  # [L 128, l 512], allowed=1
            nz = np.flatnonzero(t.any(axis=0))
            lo = min(int(nz[0]) if len(nz) else 0, 384)
            key = t.tobytes()
            if key not in uniq:
                uniq[key] = len(tiles)
                tiles.append(t)
            row.append((j, uniq[key], lo))
        if len(row) % 2:
            raise NotImplementedError("odd key-tile count not supported")
        schedule.append(row)
    if not tiles:
        tiles.append(np.ones((128, 512), np.float32))
    return schedule, np.stack(tiles)


def make_core_inputs(x, w_in, w_out, mask_tiles, b, hg):
    """Inputs for core handling batch b, heads hg*4..hg*4+3."""
    import ml_dtypes
    bf = ml_dtypes.bfloat16
    heads = range(hg * 4, hg * 4 + 4)
    xt = np.ascontiguousarray(x[b].T)
    wqk = np.concatenate(
        [w_in[:, h * 384 + o:h * 384 + o + 128] for h in heads for o in (0, 128)],
        axis=1)
    wv = np.concatenate([w_in[:, h * 384 + 256:h * 384 + 384] for h in heads], axis=1)
    wout = np.concatenate([w_out[h * 128:(h + 1) * 128, :] for h in heads], axis=0)
    return {
        "xt": np.ascontiguousarray(xt.astype(bf)),
        "wqk": np.ascontiguousarray(wqk.astype(bf)),
        "wv": np.ascontiguousarray(wv.astype(bf)),
        "wout": np.ascontiguousarray(wout.astype(bf)),
        "maskt": np.ascontiguousarray(mask_tiles.astype(bf)),
        "ones": np.ones((128, 128), bf),
    }


_CACHE = {}


def _get_nc(schedule, n_masks):
    key = (tuple(tuple(r) for r in schedule), n_masks)
    if key not in _CACHE:
        _CACHE[key] = build_nc(schedule, n_masks)
    return _CACHE[key]


def kernel(x, w_in, w_out, mask):
    """Full-input entry point: shards across 8 NeuronCores (batch x head-group),
    runs the Bass kernel SPMD, and reduces the per-core partial outputs."""
    from concourse import bass_utils
    x = np.ascontiguousarray(np.asarray(x), dtype=np.float32)
    w_in = np.ascontiguousarray(np.asarray(w_in), dtype=np.float32)
    w_out = np.ascontiguousarray(np.asarray(w_out), dtype=np.float32)
    B = x.shape[0]
    m2 = np.asarray(mask).reshape(S, S)
    schedule, mask_tiles = derive_schedule(m2)
    nc = _get_nc(schedule, mask_tiles.shape[0])
    in_maps = [make_core_inputs(x, w_in, w_out, mask_tiles, c // 4, c % 4)
               for c in range(8)]
    res = bass_utils.run_bass_kernel_spmd(nc, in_maps, core_ids=list(range(8)))
    y = np.zeros((B, S, DM), np.float32)
    for c in range(8):
        y[c // 4] += np.asarray(res.results[c]["y"], dtype=np.float32)
    return y
